# revision 31
# baseline (speedup 1.0000x reference)
"""Causal attention (B=4, S=2048, D=1024) on 8 Trainium2 NeuronCores.

Sharding: data-parallel over batch (4) x query-block-parallel (2 cores per
batch).  Global q-tiles (128 rows each, 16 per batch) are dealt round-robin:
core h=0 of a pair takes even tiles, h=1 odd tiles.  The program rounds every
q-tile's causal key-extent up to a multiple of 256 -- tile pair (2j, 2j+1)
then shares the extent 256*(j+1), so both cores run the *same* instruction
stream (SPMD) and the residual causal masking is supplied as a per-core
additive-mask input.

Reassociated algebra (cuts per-core matmul work 15.5 -> 11.1 GFLOP):
  scores = (x Wq)(x Wk)^T / 32 = x A x^T   with A = (Wq/32) Wk^T
  P V    = P (x Wv) = (P x) Wv
so the K/V projections over the full (pair-duplicated) sequence are replaced
by the once-per-core A (d x d) and per-query-block (P x) Wv products:
  A      [d',d]  = sum_e wq[d',e]/32 wk[d,e]          (128 MMs @ N=512)
  R^T    [d,q]   = sum_d' A[d',d] xq[q,d']            (128 MMs)
  S      [q,k]   = sum_d R^T[d,q]^T x[k,d]            (144 MMs)
  P      = exp(S + mask), row sums via activation accum_out
  Px     [q,d]   = sum_k P^T[k,q]^T x[k,d]            (144 MMs)
  O      [q,e]   = sum_d Px^T[d,q]^T wv[d,e] / rowsum (128 MMs)

Device transposes run on the DMA engines via the XBAR DMA-transpose, one
BATCHED instruction per tensor per j (a 3D SBUF destination [128, kt, n]
extends the logical partition dim, so a full [128, ext] P transposes in one
instruction).  Per-instruction XBAR overhead is ~1.2us and a
DmaTranspose<->DmaCopy transition serializes the global DMA stream (known
HW bug), so: only the unavoidable mid-kernel transposes (P^T, Px^T) are
XBARs (on the otherwise-empty SP queue); all input layouts that are known
up front (wq^T/32, wk^T, x^T, xq^T, plus x natural and wv) are prepared
host-side in bf16 and DMA'd as plain copies on the gpsimd/ACT queues,
weights first (they gate the A phase).  Output stores are bf16 on gpsimd.
The attention j-loop is software-pipelined with a 2-stage skew per stage
(S(s) | Px(s-2) | O(s-4)) so the PE never waits on exp->XBAR->matmul
chains; j=0 runs first so the drain ends on mid-size j=4.
PSUM-evictions run on the DVE; exp runs per 512-chunk on ACT with
accum_out providing softmax row-sums for free.
"""

import os

os.environ.setdefault("MYCRO_LOCAL_CACHE", "1")

import ml_dtypes
import numpy as np

import concourse.bacc as bacc
import concourse.tile as tile
from concourse import mybir
from concourse.bass_utils import run_bass_kernel_spmd

B, S, D = 4, 2048, 1024
P = 128
QL = S // 2          # queries per core
NCORES = 8
DT = D // P          # 8 d-tiles
ST = S // P          # 16 s-tiles
NQT = QL // P        # 8 q-tiles per core
F32 = mybir.dt.float32
BF16 = mybir.dt.bfloat16
NEG = -30000.0       # additive mask value; exp() underflows to exactly 0


def _chunks(extent):
    out, o = [], 0
    while o < extent:
        w = min(512, extent - o)
        out.append((o, w))
        o += w
    return out


def _body(tc, x, xt, xqt, wqt, wkt, wv, mask, out):
    nc = tc.nc
    with (
        tc.tile_pool(name="consts", bufs=1) as consts,
        tc.tile_pool(name="main", bufs=1) as main,
        tc.tile_pool(name="pmm", bufs=4, space="PSUM") as pmm,
        tc.tile_pool(name="psO", bufs=2, space="PSUM") as psO,
    ):
        mask_sb = consts.tile([P, 256], F32)

        xT = main.tile([P, DT, S], BF16)     # [d_in, d_tile, s]
        xqT = main.tile([P, DT, QL], BF16)   # [d_in, d_tile, q]
        xn = main.tile([P, ST, D], BF16)     # [s_in, s_tile, d]
        qa = main.tile([P, DT, QL], BF16)    # R^T = (xq A)^T : [d_in, d_tile, q]
        wv_sb = main.tile([P, DT, D], BF16)  # [d_in, d_tile, e]

        with tc.tile_pool(name="wscope", bufs=1) as ws:
            wqt_sb = ws.tile([P, DT, D], BF16)   # [e_in, e_tile, d']
            wkt_sb = ws.tile([P, DT, D], BF16)   # [e_in, e_tile, d]
            a_sb = ws.tile([P, DT, D], BF16)     # A: [d'_in, d'_tile, d]

            # All bulk input movement is plain copies (a DmaTranspose <->
            # DmaCopy transition serializes the whole DMA stream, so the only
            # device transposes are the per-j P^T/Px^T XBARs in attention).
            # Issue alternates gpsimd/scalar so descriptor-gen pipelines.
            # Order = need order: weights (gate A) -> xq^T -> x^T -> rest.
            def ld3(eng, dst, src):
                eng.dma_start(dst, src.rearrange("(t p) f -> p t f", p=P))

            for g in range(8):
                ld3(nc.gpsimd, wqt_sb[:, g:g + 1, :],
                    wqt[g * P:(g + 1) * P, :])
                ld3(nc.scalar, wkt_sb[:, g:g + 1, :],
                    wkt[g * P:(g + 1) * P, :])
            for g in range(2):
                ld3(nc.scalar, xqT[:, 4 * g:4 * g + 4, :],
                    xqt[g * 512:(g + 1) * 512, :])
            for g in range(4):
                ld3(nc.gpsimd, xT[:, 2 * g:2 * g + 2, :],
                    xt[g * 256:(g + 1) * 256, :])
            nc.scalar.dma_start(mask_sb, mask)
            for g in range(4):
                ld3((nc.gpsimd, nc.scalar)[g % 2], xn[:, 4 * g:4 * g + 4, :],
                    x[g * 512:(g + 1) * 512, :])
            for g in range(2):
                ld3((nc.gpsimd, nc.scalar)[g % 2], wv_sb[:, 4 * g:4 * g + 4, :],
                    wv[g * 512:(g + 1) * 512, :])

            # ---- HAM warmup: scratch matmuls fill the ~5us weight-load
            # window and lift the PE clock gate to 8/8 before real work
            scr = ws.tile([P, 512], BF16)
            nc.gpsimd.memset(scr, 0)
            for _ in range(20):
                pw = pmm.tile([P, 512], F32, tag="mm")
                nc.tensor.matmul(pw, scr[:, :P], scr, start=True, stop=True)

            # ---- A[d'tile, d] = sum_e wq[d',e]/32 wk[d,e]
            for ch in range(2):
                for dtp in range(DT):
                    ps = pmm.tile([P, 512], F32, tag="mm")
                    for et in range(DT):
                        nc.tensor.matmul(
                            ps, wqt_sb[:, et, dtp * P:(dtp + 1) * P],
                            wkt_sb[:, et, ch * 512:(ch + 1) * 512],
                            start=(et == 0), stop=(et == DT - 1))
                    nc.vector.tensor_copy(
                        a_sb[:, dtp, ch * 512:(ch + 1) * 512], ps)

            # ---- R^T[dtile, q] = sum_d' A[d', d] xq[q, d']
            for qc in (0, 1):        # qc=0 first: scores start at j=0
                for dt in range(DT):
                    ps = pmm.tile([P, 512], F32, tag="mm")
                    for dtp in range(DT):
                        nc.tensor.matmul(
                            ps, a_sb[:, dtp, dt * P:(dt + 1) * P],
                            xqT[:, dtp, qc * 512:(qc + 1) * 512],
                            start=(dtp == 0), stop=(dtp == DT - 1))
                    nc.vector.tensor_copy(
                        qa[:, dt, qc * 512:(qc + 1) * 512], ps)

        # ------------------------------ attention --------------------------
        with (
            tc.tile_pool(name="pp", bufs=3) as pp,
            tc.tile_pool(name="ptp", bufs=3) as ptp,
            tc.tile_pool(name="pxp", bufs=2) as pxp,
            tc.tile_pool(name="pxtp", bufs=3) as pxtp,
            tc.tile_pool(name="op", bufs=2) as op,
            tc.tile_pool(name="stats", bufs=5) as spool,
        ):
            st = {}

            def emit_scores(j):
                ext = 256 * (j + 1)
                nchunk = len(_chunks(ext))
                p_sb = pp.tile([P, S], BF16, tag="p")
                pt = ptp.tile([P, ST, P], BF16, tag="pt")
                lsum = spool.tile([P, 4], F32, tag="lsum")
                for ci, (o, w) in enumerate(_chunks(ext)):
                    ps = pmm.tile([P, 512], F32, tag="mm")
                    for dt in range(DT):
                        nc.tensor.matmul(
                            ps[:, :w], qa[:, dt, j * P:(j + 1) * P],
                            xT[:, dt, o:o + w],
                            start=(dt == 0), stop=(dt == DT - 1))
                    if o + w == ext:
                        nc.vector.tensor_add(
                            ps[:, w - 256:w], ps[:, w - 256:w], mask_sb)
                    nc.scalar.activation(
                        p_sb[:, o:o + w], ps[:, :w],
                        mybir.ActivationFunctionType.Exp,
                        accum_out=lsum[:, ci:ci + 1])
                nc.sync.dma_start(pt[:, :ext // P, :], p_sb[:, :ext],
                                  transpose=True)
                l_ = spool.tile([P, 1], F32, tag="l")
                nc.vector.reduce_sum(l_, lsum[:, :nchunk],
                                     axis=mybir.AxisListType.X)
                linv = spool.tile([P, 1], F32, tag="linv")
                nc.vector.reciprocal(linv, l_)
                st[j] = {"pt": pt, "linv": linv}

            def emit_px(j):
                nk = 2 * (j + 1)
                pt = st[j]["pt"]
                px_sb = pxp.tile([P, D], BF16, tag="px")
                pxt = pxtp.tile([P, DT, P], BF16, tag="pxt")
                for ec in range(2):
                    ps = pmm.tile([P, 512], F32, tag="mm")
                    for kt in range(nk):
                        nc.tensor.matmul(
                            ps, pt[:, kt, :], xn[:, kt, ec * 512:(ec + 1) * 512],
                            start=(kt == 0), stop=(kt == nk - 1))
                    nc.vector.tensor_copy(px_sb[:, ec * 512:(ec + 1) * 512], ps)
                nc.sync.dma_start(pxt[:, :, :], px_sb[:, :], transpose=True)
                st[j]["pxt"] = pxt

            def emit_o(j, nsplit=2):
                pxt = st[j]["pxt"]
                linv = st[j]["linv"]
                po = psO.tile([P, D], F32, tag="o")
                for ec in range(2):
                    for dt in range(DT):
                        nc.tensor.matmul(
                            po[:, ec * 512:(ec + 1) * 512], pxt[:, dt, :],
                            wv_sb[:, dt, ec * 512:(ec + 1) * 512],
                            start=(dt == 0), stop=(dt == DT - 1))
                o_sb = op.tile([P, D], BF16, tag="osb")
                w = D // nsplit
                for pc in range(nsplit):
                    nc.vector.tensor_scalar_mul(
                        o_sb[:, pc * w:(pc + 1) * w],
                        po[:, pc * w:(pc + 1) * w], linv)
                    nc.gpsimd.dma_start(
                        out[j * P:(j + 1) * P, pc * w:(pc + 1) * w],
                        o_sb[:, pc * w:(pc + 1) * w])
                del st[j]

            # j=0 first so the pipeline drains on a mid-size j (its long Px
            # hides the final PxT XBAR latency); big/small interleave keeps
            # short stages' serialized PT/PxT/out DMA chains hidden.
            # skew-2 between stages: every XBAR gets two stages to land.
            js = [0, 7, 3, 6, 2, 5, 1, 4]
            for step in range(len(js) + 4):
                if step < len(js):
                    emit_scores(js[step])
                if 2 <= step < len(js) + 2:
                    emit_px(js[step - 2])
                if step >= 4:
                    emit_o(js[step - 4], nsplit=4 if step == len(js) + 3 else 2)


_PROG = None


def _get_prog():
    global _PROG
    if _PROG is None:
        nc = bacc.Bacc("TRN2", target_bir_lowering=False, debug=False,
                       enable_asserts=False)
        x = nc.dram_tensor("x", (S, D), BF16, kind="ExternalInput").ap()
        xt = nc.dram_tensor("xt", (D, S), BF16, kind="ExternalInput").ap()
        xqt = nc.dram_tensor("xqt", (D, QL), BF16, kind="ExternalInput").ap()
        wqt = nc.dram_tensor("wqt", (D, D), BF16, kind="ExternalInput").ap()
        wkt = nc.dram_tensor("wkt", (D, D), BF16, kind="ExternalInput").ap()
        wv = nc.dram_tensor("wv", (D, D), BF16, kind="ExternalInput").ap()
        mask = nc.dram_tensor("mask", (P, 256), F32, kind="ExternalInput").ap()
        out = nc.dram_tensor("out", (QL, D), BF16, kind="ExternalOutput").ap()
        with tile.TileContext(nc) as tc:
            _body(tc, x, xt, xqt, wqt, wkt, wv, mask, out)
        nc.compile()
        _PROG = nc
    return _PROG


def _mask_np(h):
    r = np.arange(P)[:, None]
    c = np.arange(P)[None, :]
    tri = np.where(c <= r, 0.0, NEG).astype(np.float32)
    m = np.zeros((P, 256), np.float32)
    if h == 0:
        m[:, :P] = tri
        m[:, P:] = NEG
    else:
        m[:, P:] = tri
    return m


def _prep_shared(inputs):
    bf = ml_dtypes.bfloat16
    wq = np.asarray(inputs["wq"], np.float32)
    wk = np.asarray(inputs["wk"], np.float32)
    wv = np.asarray(inputs["wv"], np.float32)
    return {
        "wqt": np.ascontiguousarray((wq / 32.0).astype(bf).T),
        "wkt": np.ascontiguousarray(wk.astype(bf).T),
        "wv": np.ascontiguousarray(wv.astype(bf)),
    }


def _in_map_for_core(inputs, core, shared=None):
    b, h = core // 2, core % 2
    if shared is None:
        shared = _prep_shared(inputs)
    xb = np.ascontiguousarray(
        np.asarray(inputs["x"], np.float32)[b].astype(ml_dtypes.bfloat16))
    xqb = xb.reshape(NQT, 2, P, D)[:, h].reshape(QL, D)
    return {"x": xb, "xt": np.ascontiguousarray(xb.T),
            "xqt": np.ascontiguousarray(xqb.T), "mask": _mask_np(h), **shared}


def _run(inputs, trace=False, tmpdir=None):
    nc = _get_prog()
    shared = _prep_shared(inputs)
    in_maps = [_in_map_for_core(inputs, c, shared) for c in range(NCORES)]
    res = None
    for attempt in range(3):
        try:
            res = run_bass_kernel_spmd(nc, in_maps,
                                       core_ids=list(range(NCORES)),
                                       trace=trace, tmpdir=tmpdir)
            break
        except Exception:
            # first execution of a fresh NEFF occasionally trips a transient
            # device error on this stack; a retry has always succeeded
            if attempt == 2:
                raise
    outf = np.empty((B, S, D), np.float32)
    for core in range(NCORES):
        b, h = core // 2, core % 2
        o = np.asarray(res.results[core]["out"], np.float32)
        outf[b].reshape(NQT, 2, P, D)[:, h] = o.reshape(NQT, P, D)
    return outf, res


def kernel(x, wq, wk, wv):
    outf, _ = _run({"x": x, "wq": wq, "wk": wk, "wv": wv}, trace=False)
    return outf


# revision 32
# speedup vs baseline: 1.0387x; 1.0387x over previous
"""Causal attention (B=4, S=2048, D=1024) on 8 Trainium2 NeuronCores.

Sharding: data-parallel over batch (4) x query-block-parallel (2 cores per
batch).  Global q-tiles (128 rows each, 16 per batch) are dealt round-robin:
core h=0 of a pair takes even tiles, h=1 odd tiles.  The program rounds every
q-tile's causal key-extent up to a multiple of 256 -- tile pair (2j, 2j+1)
then shares the extent 256*(j+1), so both cores run the *same* instruction
stream (SPMD) and the residual causal masking is supplied as a per-core
additive-mask input.

Reassociated algebra (cuts per-core matmul work 15.5 -> 11.1 GFLOP):
  scores = (x Wq)(x Wk)^T / 32 = x A x^T   with A = (Wq/32) Wk^T
  P V    = P (x Wv) = (P x) Wv
so the K/V projections over the full (pair-duplicated) sequence are replaced
by the once-per-core A (d x d) and per-query-block (P x) Wv products:
  A      [d',d]  = sum_e wq[d',e]/32 wk[d,e]          (128 MMs @ N=512)
  R^T    [d,q]   = sum_d' A[d',d] xq[q,d']            (128 MMs)
  S      [q,k]   = sum_d R^T[d,q]^T x[k,d]            (144 MMs)
  P      = exp(S + mask), row sums via activation accum_out
  Px     [q,d]   = sum_k P^T[k,q]^T x[k,d]            (144 MMs)
  O      [q,e]   = sum_d Px^T[d,q]^T wv[d,e] / rowsum (128 MMs)

Device transposes run on the DMA engines via the XBAR DMA-transpose, one
BATCHED instruction per tensor per j (a 3D SBUF destination [128, kt, n]
extends the logical partition dim, so a full [128, ext] P transposes in one
instruction).  Per-instruction XBAR overhead is ~1.2us and a
DmaTranspose<->DmaCopy transition serializes the global DMA stream (known
HW bug), so: only the unavoidable mid-kernel transposes (P^T, Px^T) are
XBARs (on the otherwise-empty SP queue); all input layouts that are known
up front (wq^T/32, wk^T, x^T, xq^T, plus x natural and wv) are prepared
host-side in bf16 and DMA'd as plain copies on the gpsimd/ACT queues,
weights first (they gate the A phase).  Output stores are bf16 on gpsimd.
The attention j-loop is software-pipelined with a 2-stage skew per stage
(S(s) | Px(s-2) | O(s-4)) so the PE never waits on exp->XBAR->matmul
chains; j=0 runs first so the drain ends on mid-size j=4.
PSUM-evictions run on the DVE; exp runs per 512-chunk on ACT with
accum_out providing softmax row-sums for free.
"""

import os

os.environ.setdefault("MYCRO_LOCAL_CACHE", "1")

import ml_dtypes
import numpy as np

import concourse.bacc as bacc
import concourse.tile as tile
from concourse import mybir
from concourse.bass_utils import run_bass_kernel_spmd

B, S, D = 4, 2048, 1024
P = 128
QL = S // 2          # queries per core
NCORES = 8
DT = D // P          # 8 d-tiles
ST = S // P          # 16 s-tiles
NQT = QL // P        # 8 q-tiles per core
F32 = mybir.dt.float32
BF16 = mybir.dt.bfloat16
NEG = -30000.0       # additive mask value; exp() underflows to exactly 0


def _chunks(extent):
    out, o = [], 0
    while o < extent:
        w = min(512, extent - o)
        out.append((o, w))
        o += w
    return out


def _body(tc, x, xt, xqt, wqt, wkt, wv, mask, out):
    nc = tc.nc
    with (
        tc.tile_pool(name="consts", bufs=1) as consts,
        tc.tile_pool(name="main", bufs=1) as main,
        tc.tile_pool(name="pmm", bufs=4, space="PSUM") as pmm,
        tc.tile_pool(name="psO", bufs=2, space="PSUM") as psO,
    ):
        mask_sb = consts.tile([P, 256], F32)

        xT = main.tile([P, DT, S], BF16)     # [d_in, d_tile, s]
        xqT = main.tile([P, DT, QL], BF16)   # [d_in, d_tile, q]
        xn = main.tile([P, ST, D], BF16)     # [s_in, s_tile, d]
        qa = main.tile([P, DT, QL], BF16)    # R^T = (xq A)^T : [d_in, d_tile, q]
        wv_sb = main.tile([P, DT, D], BF16)  # [d_in, d_tile, e]

        with tc.tile_pool(name="wscope", bufs=1) as ws:
            wqt_sb = ws.tile([P, DT, D], BF16)   # [e_in, e_tile, d']
            wkt_sb = ws.tile([P, DT, D], BF16)   # [e_in, e_tile, d]
            a_sb = ws.tile([P, DT, D], BF16)     # A: [d'_in, d'_tile, d]

            # All bulk input movement is plain copies (a DmaTranspose <->
            # DmaCopy transition serializes the whole DMA stream, so the only
            # device transposes are the per-j P^T/Px^T XBARs in attention).
            # Issue alternates gpsimd/scalar so descriptor-gen pipelines.
            # Order = need order: weights (gate A) -> xq^T -> x^T -> rest.
            def ld3(eng, dst, src):
                eng.dma_start(dst, src.rearrange("(t p) f -> p t f", p=P))

            for g in range(8):
                ld3(nc.gpsimd, wqt_sb[:, g:g + 1, :],
                    wqt[g * P:(g + 1) * P, :])
                ld3(nc.scalar, wkt_sb[:, g:g + 1, :],
                    wkt[g * P:(g + 1) * P, :])
            for g in range(2):
                ld3(nc.scalar, xqT[:, 4 * g:4 * g + 4, :],
                    xqt[g * 512:(g + 1) * 512, :])
            for g in range(4):
                ld3(nc.gpsimd, xT[:, 2 * g:2 * g + 2, :],
                    xt[g * 256:(g + 1) * 256, :])
            nc.scalar.dma_start(mask_sb, mask)
            for g in range(4):
                ld3((nc.gpsimd, nc.scalar)[g % 2], xn[:, 4 * g:4 * g + 4, :],
                    x[g * 512:(g + 1) * 512, :])
            for g in range(2):
                ld3((nc.gpsimd, nc.scalar)[g % 2], wv_sb[:, 4 * g:4 * g + 4, :],
                    wv[g * 512:(g + 1) * 512, :])


            # ---- A[d'tile, d] = sum_e wq[d',e]/32 wk[d,e]
            for ch in range(2):
                for dtp in range(DT):
                    ps = pmm.tile([P, 512], F32, tag="mm")
                    for et in range(DT):
                        nc.tensor.matmul(
                            ps, wqt_sb[:, et, dtp * P:(dtp + 1) * P],
                            wkt_sb[:, et, ch * 512:(ch + 1) * 512],
                            start=(et == 0), stop=(et == DT - 1))
                    nc.vector.tensor_copy(
                        a_sb[:, dtp, ch * 512:(ch + 1) * 512], ps)

            # ---- R^T[dtile, q] = sum_d' A[d', d] xq[q, d']
            for qc in (0, 1):        # qc=0 first: scores start at j=0
                for dt in range(DT):
                    ps = pmm.tile([P, 512], F32, tag="mm")
                    for dtp in range(DT):
                        nc.tensor.matmul(
                            ps, a_sb[:, dtp, dt * P:(dt + 1) * P],
                            xqT[:, dtp, qc * 512:(qc + 1) * 512],
                            start=(dtp == 0), stop=(dtp == DT - 1))
                    nc.vector.tensor_copy(
                        qa[:, dt, qc * 512:(qc + 1) * 512], ps)

        # ------------------------------ attention --------------------------
        with (
            tc.tile_pool(name="pp", bufs=3) as pp,
            tc.tile_pool(name="ptp", bufs=3) as ptp,
            tc.tile_pool(name="pxp", bufs=2) as pxp,
            tc.tile_pool(name="pxtp", bufs=3) as pxtp,
            tc.tile_pool(name="op", bufs=2) as op,
            tc.tile_pool(name="stats", bufs=5) as spool,
        ):
            st = {}

            def emit_scores(j):
                ext = 256 * (j + 1)
                nchunk = len(_chunks(ext))
                p_sb = pp.tile([P, S], BF16, tag="p")
                pt = ptp.tile([P, ST, P], BF16, tag="pt")
                lsum = spool.tile([P, 4], F32, tag="lsum")
                for ci, (o, w) in enumerate(_chunks(ext)):
                    ps = pmm.tile([P, 512], F32, tag="mm")
                    for dt in range(DT):
                        nc.tensor.matmul(
                            ps[:, :w], qa[:, dt, j * P:(j + 1) * P],
                            xT[:, dt, o:o + w],
                            start=(dt == 0), stop=(dt == DT - 1))
                    if o + w == ext:
                        nc.vector.tensor_add(
                            ps[:, w - 256:w], ps[:, w - 256:w], mask_sb)
                    nc.scalar.activation(
                        p_sb[:, o:o + w], ps[:, :w],
                        mybir.ActivationFunctionType.Exp,
                        accum_out=lsum[:, ci:ci + 1])
                nc.sync.dma_start(pt[:, :ext // P, :], p_sb[:, :ext],
                                  transpose=True)
                l_ = spool.tile([P, 1], F32, tag="l")
                nc.vector.reduce_sum(l_, lsum[:, :nchunk],
                                     axis=mybir.AxisListType.X)
                linv = spool.tile([P, 1], F32, tag="linv")
                nc.vector.reciprocal(linv, l_)
                st[j] = {"pt": pt, "linv": linv}

            def emit_px(j):
                nk = 2 * (j + 1)
                pt = st[j]["pt"]
                px_sb = pxp.tile([P, D], BF16, tag="px")
                pxt = pxtp.tile([P, DT, P], BF16, tag="pxt")
                for ec in range(2):
                    ps = pmm.tile([P, 512], F32, tag="mm")
                    for kt in range(nk):
                        nc.tensor.matmul(
                            ps, pt[:, kt, :], xn[:, kt, ec * 512:(ec + 1) * 512],
                            start=(kt == 0), stop=(kt == nk - 1))
                    nc.vector.tensor_copy(px_sb[:, ec * 512:(ec + 1) * 512], ps)
                nc.sync.dma_start(pxt[:, :, :], px_sb[:, :], transpose=True)
                st[j]["pxt"] = pxt

            def emit_o(j, nsplit=2):
                pxt = st[j]["pxt"]
                linv = st[j]["linv"]
                po = psO.tile([P, D], F32, tag="o")
                for ec in range(2):
                    for dt in range(DT):
                        nc.tensor.matmul(
                            po[:, ec * 512:(ec + 1) * 512], pxt[:, dt, :],
                            wv_sb[:, dt, ec * 512:(ec + 1) * 512],
                            start=(dt == 0), stop=(dt == DT - 1))
                o_sb = op.tile([P, D], BF16, tag="osb")
                w = D // nsplit
                for pc in range(nsplit):
                    nc.vector.tensor_scalar_mul(
                        o_sb[:, pc * w:(pc + 1) * w],
                        po[:, pc * w:(pc + 1) * w], linv)
                    nc.gpsimd.dma_start(
                        out[j * P:(j + 1) * P, pc * w:(pc + 1) * w],
                        o_sb[:, pc * w:(pc + 1) * w])
                del st[j]

            # j=0 first so the pipeline drains on a mid-size j (its long Px
            # hides the final PxT XBAR latency); big/small interleave keeps
            # short stages' serialized PT/PxT/out DMA chains hidden.
            # skew-2 between stages: every XBAR gets two stages to land.
            js = [0, 7, 3, 6, 2, 5, 1, 4]
            for step in range(len(js) + 4):
                if step < len(js):
                    emit_scores(js[step])
                if 2 <= step < len(js) + 2:
                    emit_px(js[step - 2])
                if step >= 4:
                    emit_o(js[step - 4], nsplit=4 if step == len(js) + 3 else 2)


_PROG = None


def _get_prog():
    global _PROG
    if _PROG is None:
        nc = bacc.Bacc("TRN2", target_bir_lowering=False, debug=False,
                       enable_asserts=False)
        x = nc.dram_tensor("x", (S, D), BF16, kind="ExternalInput").ap()
        xt = nc.dram_tensor("xt", (D, S), BF16, kind="ExternalInput").ap()
        xqt = nc.dram_tensor("xqt", (D, QL), BF16, kind="ExternalInput").ap()
        wqt = nc.dram_tensor("wqt", (D, D), BF16, kind="ExternalInput").ap()
        wkt = nc.dram_tensor("wkt", (D, D), BF16, kind="ExternalInput").ap()
        wv = nc.dram_tensor("wv", (D, D), BF16, kind="ExternalInput").ap()
        mask = nc.dram_tensor("mask", (P, 256), F32, kind="ExternalInput").ap()
        out = nc.dram_tensor("out", (QL, D), BF16, kind="ExternalOutput").ap()
        with tile.TileContext(nc) as tc:
            _body(tc, x, xt, xqt, wqt, wkt, wv, mask, out)
        nc.compile()
        _PROG = nc
    return _PROG


def _mask_np(h):
    r = np.arange(P)[:, None]
    c = np.arange(P)[None, :]
    tri = np.where(c <= r, 0.0, NEG).astype(np.float32)
    m = np.zeros((P, 256), np.float32)
    if h == 0:
        m[:, :P] = tri
        m[:, P:] = NEG
    else:
        m[:, P:] = tri
    return m


def _prep_shared(inputs):
    bf = ml_dtypes.bfloat16
    wq = np.asarray(inputs["wq"], np.float32)
    wk = np.asarray(inputs["wk"], np.float32)
    wv = np.asarray(inputs["wv"], np.float32)
    return {
        "wqt": np.ascontiguousarray((wq / 32.0).astype(bf).T),
        "wkt": np.ascontiguousarray(wk.astype(bf).T),
        "wv": np.ascontiguousarray(wv.astype(bf)),
    }


def _in_map_for_core(inputs, core, shared=None):
    b, h = core // 2, core % 2
    if shared is None:
        shared = _prep_shared(inputs)
    xb = np.ascontiguousarray(
        np.asarray(inputs["x"], np.float32)[b].astype(ml_dtypes.bfloat16))
    xqb = xb.reshape(NQT, 2, P, D)[:, h].reshape(QL, D)
    return {"x": xb, "xt": np.ascontiguousarray(xb.T),
            "xqt": np.ascontiguousarray(xqb.T), "mask": _mask_np(h), **shared}


def _run(inputs, trace=False, tmpdir=None):
    nc = _get_prog()
    shared = _prep_shared(inputs)
    in_maps = [_in_map_for_core(inputs, c, shared) for c in range(NCORES)]
    res = None
    for attempt in range(3):
        try:
            res = run_bass_kernel_spmd(nc, in_maps,
                                       core_ids=list(range(NCORES)),
                                       trace=trace, tmpdir=tmpdir)
            break
        except Exception:
            # first execution of a fresh NEFF occasionally trips a transient
            # device error on this stack; a retry has always succeeded
            if attempt == 2:
                raise
    outf = np.empty((B, S, D), np.float32)
    for core in range(NCORES):
        b, h = core // 2, core % 2
        o = np.asarray(res.results[core]["out"], np.float32)
        outf[b].reshape(NQT, 2, P, D)[:, h] = o.reshape(NQT, P, D)
    return outf, res


def kernel(x, wq, wk, wv):
    outf, _ = _run({"x": x, "wq": wq, "wk": wk, "wv": wv}, trace=False)
    return outf


# revision 34
# speedup vs baseline: 1.0532x; 1.0139x over previous
"""Causal attention (B=4, S=2048, D=1024) on 8 Trainium2 NeuronCores.

Sharding: data-parallel over batch (4) x query-block-parallel (2 cores per
batch).  Global q-tiles (128 rows each, 16 per batch) are dealt round-robin:
core h=0 of a pair takes even tiles, h=1 odd tiles.  The program rounds every
q-tile's causal key-extent up to a multiple of 256 -- tile pair (2j, 2j+1)
then shares the extent 256*(j+1), so both cores run the *same* instruction
stream (SPMD) and the residual causal masking is supplied as a per-core
additive-mask input.

Reassociated algebra (cuts per-core matmul work 15.5 -> 11.1 GFLOP):
  scores = (x Wq)(x Wk)^T / 32 = x A x^T   with A = (Wq/32) Wk^T
  P V    = P (x Wv) = (P x) Wv
so the K/V projections over the full (pair-duplicated) sequence are replaced
by the once-per-core A (d x d) and per-query-block (P x) Wv products:
  A      [d',d]  = sum_e wq[d',e]/32 wk[d,e]          (128 MMs @ N=512)
  R^T    [d,q]   = sum_d' A[d',d] xq[q,d']            (128 MMs)
  S      [q,k]   = sum_d R^T[d,q]^T x[k,d]            (144 MMs)
  P      = exp(S + mask), row sums via activation accum_out
  Px     [q,d]   = sum_k P^T[k,q]^T x[k,d]            (144 MMs)
  O      [q,e]   = sum_d Px^T[d,q]^T wv[d,e] / rowsum (128 MMs)

Device transposes run on the DMA engines via the XBAR DMA-transpose, one
BATCHED instruction per tensor per j (a 3D SBUF destination [128, kt, n]
extends the logical partition dim, so a full [128, ext] P transposes in one
instruction).  Per-instruction XBAR overhead is ~1.2us and a
DmaTranspose<->DmaCopy transition serializes the global DMA stream (known
HW bug), so: only the unavoidable mid-kernel transposes (P^T, Px^T) are
XBARs (on the otherwise-empty SP queue); all input layouts that are known
up front (wq^T/32, wk^T, x^T, xq^T, plus x natural and wv) are prepared
host-side in bf16 and DMA'd as plain copies on the gpsimd/ACT queues,
weights first (they gate the A phase).  Output stores are bf16 on gpsimd.
The attention j-loop is software-pipelined with a 2-stage skew per stage
(S(s) | Px(s-2) | O(s-4)) so the PE never waits on exp->XBAR->matmul
chains; j=0 runs first so the drain ends on mid-size j=4.
PSUM-evictions run on the DVE; exp runs per 512-chunk on ACT with
accum_out providing softmax row-sums for free.
"""

import os

os.environ.setdefault("MYCRO_LOCAL_CACHE", "1")

import ml_dtypes
import numpy as np

import concourse.bacc as bacc
import concourse.tile as tile
from concourse import mybir
from concourse.bass_utils import run_bass_kernel_spmd

B, S, D = 4, 2048, 1024
P = 128
QL = S // 2          # queries per core
NCORES = 8
DT = D // P          # 8 d-tiles
ST = S // P          # 16 s-tiles
NQT = QL // P        # 8 q-tiles per core
F32 = mybir.dt.float32
BF16 = mybir.dt.bfloat16
NEG = -30000.0       # additive mask value; exp() underflows to exactly 0


def _chunks(extent):
    out, o = [], 0
    while o < extent:
        w = min(512, extent - o)
        out.append((o, w))
        o += w
    return out


def _body(tc, x, xt, xqt, wqt, wkt, wv, mask, out):
    nc = tc.nc
    with (
        tc.tile_pool(name="consts", bufs=1) as consts,
        tc.tile_pool(name="main", bufs=1) as main,
        tc.tile_pool(name="pmm", bufs=4, space="PSUM") as pmm,
        tc.tile_pool(name="psO", bufs=2, space="PSUM") as psO,
    ):
        mask_sb = consts.tile([P, 256], F32)

        xT = main.tile([P, DT, S], BF16)     # [d_in, d_tile, s]
        xqT = main.tile([P, DT, QL], BF16)   # [d_in, d_tile, q]
        xn = main.tile([P, ST, D], BF16)     # [s_in, s_tile, d]
        qa = main.tile([P, DT, QL], BF16)    # R^T = (xq A)^T : [d_in, d_tile, q]
        wv_sb = main.tile([P, DT, D], BF16)  # [d_in, d_tile, e]

        with tc.tile_pool(name="wscope", bufs=1) as ws:
            wqt_sb = ws.tile([P, DT, D], BF16)   # [e_in, e_tile, d']
            wkt_sb = ws.tile([P, DT, D], BF16)   # [e_in, e_tile, d]
            a_sb = ws.tile([P, DT, D], BF16)     # A: [d'_in, d'_tile, d]

            # All bulk input movement is plain copies (a DmaTranspose <->
            # DmaCopy transition serializes the whole DMA stream, so the only
            # device transposes are the per-j P^T/Px^T XBARs in attention).
            # Issue alternates gpsimd/scalar so descriptor-gen pipelines.
            # Order = need order: weights (gate A) -> xq^T -> x^T -> rest.
            def ld3(eng, dst, src):
                eng.dma_start(dst, src.rearrange("(t p) f -> p t f", p=P))

            for g in range(8):
                ld3(nc.gpsimd, wqt_sb[:, g:g + 1, :],
                    wqt[g * P:(g + 1) * P, :])
                ld3(nc.scalar, wkt_sb[:, g:g + 1, :],
                    wkt[g * P:(g + 1) * P, :])
            for g in range(2):
                ld3(nc.scalar, xqT[:, 4 * g:4 * g + 4, :],
                    xqt[g * 512:(g + 1) * 512, :])
            for g in range(4):
                ld3(nc.gpsimd, xT[:, 2 * g:2 * g + 2, :],
                    xt[g * 256:(g + 1) * 256, :])
            nc.scalar.dma_start(mask_sb, mask)
            for g in range(4):
                ld3((nc.gpsimd, nc.scalar)[g % 2], xn[:, 4 * g:4 * g + 4, :],
                    x[g * 512:(g + 1) * 512, :])
            for g in range(2):
                ld3((nc.gpsimd, nc.scalar)[g % 2], wv_sb[:, 4 * g:4 * g + 4, :],
                    wv[g * 512:(g + 1) * 512, :])

            # ---- HAM warmup: scratch matmuls (DVE-memset source, so they
            # start ~5us in) fill the weight-load window and lift the PE
            # clock gate to 8/8 before the real stream begins
            scr = consts.tile([P, 512], BF16)
            nc.vector.memset(scr, 0)
            for _ in range(12):
                pw = pmm.tile([P, 512], F32, tag="mm")
                nc.tensor.matmul(pw, scr[:, :P], scr, start=True, stop=True)

            # ---- A[d'tile, d] = sum_e wq[d',e]/32 wk[d,e]
            for ch in range(2):
                for dtp in range(DT):
                    ps = pmm.tile([P, 512], F32, tag="mm")
                    for et in range(DT):
                        nc.tensor.matmul(
                            ps, wqt_sb[:, et, dtp * P:(dtp + 1) * P],
                            wkt_sb[:, et, ch * 512:(ch + 1) * 512],
                            start=(et == 0), stop=(et == DT - 1))
                    nc.vector.tensor_copy(
                        a_sb[:, dtp, ch * 512:(ch + 1) * 512], ps)

            # ---- R^T[dtile, q] = sum_d' A[d', d] xq[q, d']
            for qc in (0, 1):        # qc=0 first: scores start at j=0
                for dt in range(DT):
                    ps = pmm.tile([P, 512], F32, tag="mm")
                    for dtp in range(DT):
                        nc.tensor.matmul(
                            ps, a_sb[:, dtp, dt * P:(dt + 1) * P],
                            xqT[:, dtp, qc * 512:(qc + 1) * 512],
                            start=(dtp == 0), stop=(dtp == DT - 1))
                    nc.vector.tensor_copy(
                        qa[:, dt, qc * 512:(qc + 1) * 512], ps)

        # ------------------------------ attention --------------------------
        with (
            tc.tile_pool(name="pp", bufs=3) as pp,
            tc.tile_pool(name="ptp", bufs=3) as ptp,
            tc.tile_pool(name="pxp", bufs=2) as pxp,
            tc.tile_pool(name="pxtp", bufs=3) as pxtp,
            tc.tile_pool(name="op", bufs=2) as op,
            tc.tile_pool(name="stats", bufs=5) as spool,
        ):
            st = {}

            def emit_scores(j):
                ext = 256 * (j + 1)
                nchunk = len(_chunks(ext))
                p_sb = pp.tile([P, S], BF16, tag="p")
                pt = ptp.tile([P, ST, P], BF16, tag="pt")
                lsum = spool.tile([P, 4], F32, tag="lsum")
                for ci, (o, w) in enumerate(_chunks(ext)):
                    ps = pmm.tile([P, 512], F32, tag="mm")
                    for dt in range(DT):
                        nc.tensor.matmul(
                            ps[:, :w], qa[:, dt, j * P:(j + 1) * P],
                            xT[:, dt, o:o + w],
                            start=(dt == 0), stop=(dt == DT - 1))
                    if o + w == ext:
                        nc.vector.tensor_add(
                            ps[:, w - 256:w], ps[:, w - 256:w], mask_sb)
                    nc.scalar.activation(
                        p_sb[:, o:o + w], ps[:, :w],
                        mybir.ActivationFunctionType.Exp,
                        accum_out=lsum[:, ci:ci + 1])
                nc.sync.dma_start(pt[:, :ext // P, :], p_sb[:, :ext],
                                  transpose=True)
                l_ = spool.tile([P, 1], F32, tag="l")
                nc.vector.reduce_sum(l_, lsum[:, :nchunk],
                                     axis=mybir.AxisListType.X)
                linv = spool.tile([P, 1], F32, tag="linv")
                nc.vector.reciprocal(linv, l_)
                st[j] = {"pt": pt, "linv": linv}

            def emit_px(j):
                nk = 2 * (j + 1)
                pt = st[j]["pt"]
                px_sb = pxp.tile([P, D], BF16, tag="px")
                pxt = pxtp.tile([P, DT, P], BF16, tag="pxt")
                for ec in range(2):
                    ps = pmm.tile([P, 512], F32, tag="mm")
                    for kt in range(nk):
                        nc.tensor.matmul(
                            ps, pt[:, kt, :], xn[:, kt, ec * 512:(ec + 1) * 512],
                            start=(kt == 0), stop=(kt == nk - 1))
                    nc.vector.tensor_copy(px_sb[:, ec * 512:(ec + 1) * 512], ps)
                nc.sync.dma_start(pxt[:, :, :], px_sb[:, :], transpose=True)
                st[j]["pxt"] = pxt

            def emit_o(j, nsplit=2):
                pxt = st[j]["pxt"]
                linv = st[j]["linv"]
                po = psO.tile([P, D], F32, tag="o")
                for ec in range(2):
                    for dt in range(DT):
                        nc.tensor.matmul(
                            po[:, ec * 512:(ec + 1) * 512], pxt[:, dt, :],
                            wv_sb[:, dt, ec * 512:(ec + 1) * 512],
                            start=(dt == 0), stop=(dt == DT - 1))
                o_sb = op.tile([P, D], BF16, tag="osb")
                w = D // nsplit
                for pc in range(nsplit):
                    nc.vector.tensor_scalar_mul(
                        o_sb[:, pc * w:(pc + 1) * w],
                        po[:, pc * w:(pc + 1) * w], linv)
                    nc.gpsimd.dma_start(
                        out[j * P:(j + 1) * P, pc * w:(pc + 1) * w],
                        o_sb[:, pc * w:(pc + 1) * w])
                del st[j]

            # j=0 first so the pipeline drains on a mid-size j (its long Px
            # hides the final PxT XBAR latency); big/small interleave keeps
            # short stages' serialized PT/PxT/out DMA chains hidden.
            # skew-2 between stages: every XBAR gets two stages to land.
            js = [0, 7, 3, 6, 2, 5, 1, 4]
            for step in range(len(js) + 4):
                if step < len(js):
                    emit_scores(js[step])
                if 2 <= step < len(js) + 2:
                    emit_px(js[step - 2])
                if step >= 4:
                    emit_o(js[step - 4], nsplit=4 if step == len(js) + 3 else 2)


_PROG = None


def _get_prog():
    global _PROG
    if _PROG is None:
        nc = bacc.Bacc("TRN2", target_bir_lowering=False, debug=False,
                       enable_asserts=False)
        x = nc.dram_tensor("x", (S, D), BF16, kind="ExternalInput").ap()
        xt = nc.dram_tensor("xt", (D, S), BF16, kind="ExternalInput").ap()
        xqt = nc.dram_tensor("xqt", (D, QL), BF16, kind="ExternalInput").ap()
        wqt = nc.dram_tensor("wqt", (D, D), BF16, kind="ExternalInput").ap()
        wkt = nc.dram_tensor("wkt", (D, D), BF16, kind="ExternalInput").ap()
        wv = nc.dram_tensor("wv", (D, D), BF16, kind="ExternalInput").ap()
        mask = nc.dram_tensor("mask", (P, 256), F32, kind="ExternalInput").ap()
        out = nc.dram_tensor("out", (QL, D), BF16, kind="ExternalOutput").ap()
        with tile.TileContext(nc) as tc:
            _body(tc, x, xt, xqt, wqt, wkt, wv, mask, out)
        nc.compile()
        _PROG = nc
    return _PROG


def _mask_np(h):
    r = np.arange(P)[:, None]
    c = np.arange(P)[None, :]
    tri = np.where(c <= r, 0.0, NEG).astype(np.float32)
    m = np.zeros((P, 256), np.float32)
    if h == 0:
        m[:, :P] = tri
        m[:, P:] = NEG
    else:
        m[:, P:] = tri
    return m


def _prep_shared(inputs):
    bf = ml_dtypes.bfloat16
    wq = np.asarray(inputs["wq"], np.float32)
    wk = np.asarray(inputs["wk"], np.float32)
    wv = np.asarray(inputs["wv"], np.float32)
    return {
        "wqt": np.ascontiguousarray((wq / 32.0).astype(bf).T),
        "wkt": np.ascontiguousarray(wk.astype(bf).T),
        "wv": np.ascontiguousarray(wv.astype(bf)),
    }


def _in_map_for_core(inputs, core, shared=None):
    b, h = core // 2, core % 2
    if shared is None:
        shared = _prep_shared(inputs)
    xb = np.ascontiguousarray(
        np.asarray(inputs["x"], np.float32)[b].astype(ml_dtypes.bfloat16))
    xqb = xb.reshape(NQT, 2, P, D)[:, h].reshape(QL, D)
    return {"x": xb, "xt": np.ascontiguousarray(xb.T),
            "xqt": np.ascontiguousarray(xqb.T), "mask": _mask_np(h), **shared}


def _run(inputs, trace=False, tmpdir=None):
    nc = _get_prog()
    shared = _prep_shared(inputs)
    in_maps = [_in_map_for_core(inputs, c, shared) for c in range(NCORES)]
    res = None
    for attempt in range(3):
        try:
            res = run_bass_kernel_spmd(nc, in_maps,
                                       core_ids=list(range(NCORES)),
                                       trace=trace, tmpdir=tmpdir)
            break
        except Exception:
            # first execution of a fresh NEFF occasionally trips a transient
            # device error on this stack; a retry has always succeeded
            if attempt == 2:
                raise
    outf = np.empty((B, S, D), np.float32)
    for core in range(NCORES):
        b, h = core // 2, core % 2
        o = np.asarray(res.results[core]["out"], np.float32)
        outf[b].reshape(NQT, 2, P, D)[:, h] = o.reshape(NQT, P, D)
    return outf, res


def kernel(x, wq, wk, wv):
    outf, _ = _run({"x": x, "wq": wq, "wk": wk, "wv": wv}, trace=False)
    return outf



# revision 35
# speedup vs baseline: 1.0578x; 1.0044x over previous
"""Causal attention (B=4, S=2048, D=1024) on 8 Trainium2 NeuronCores.

Sharding: data-parallel over batch (4) x query-block-parallel (2 cores per
batch).  Global q-tiles (128 rows each, 16 per batch) are dealt round-robin:
core h=0 of a pair takes even tiles, h=1 odd tiles.  The program rounds every
q-tile's causal key-extent up to a multiple of 256 -- tile pair (2j, 2j+1)
then shares the extent 256*(j+1), so both cores run the *same* instruction
stream (SPMD) and the residual causal masking is supplied as a per-core
additive-mask input.

Reassociated algebra (cuts per-core matmul work 15.5 -> 11.1 GFLOP):
  scores = (x Wq)(x Wk)^T / 32 = x A x^T   with A = (Wq/32) Wk^T
  P V    = P (x Wv) = (P x) Wv
so the K/V projections over the full (pair-duplicated) sequence are replaced
by the once-per-core A (d x d) and per-query-block (P x) Wv products:
  A      [d',d]  = sum_e wq[d',e]/32 wk[d,e]          (128 MMs @ N=512)
  R^T    [d,q]   = sum_d' A[d',d] xq[q,d']            (128 MMs)
  S      [q,k]   = sum_d R^T[d,q]^T x[k,d]            (144 MMs)
  P      = exp(S + mask), row sums via activation accum_out
  Px     [q,d]   = sum_k P^T[k,q]^T x[k,d]            (144 MMs)
  O      [q,e]   = sum_d Px^T[d,q]^T wv[d,e] / rowsum (128 MMs)

Device transposes run on the DMA engines via the XBAR DMA-transpose, one
BATCHED instruction per tensor per j (a 3D SBUF destination [128, kt, n]
extends the logical partition dim, so a full [128, ext] P transposes in one
instruction).  Per-instruction XBAR overhead is ~1.2us and a
DmaTranspose<->DmaCopy transition serializes the global DMA stream (known
HW bug), so: only the unavoidable mid-kernel transposes (P^T, Px^T) are
XBARs (on the otherwise-empty SP queue); all input layouts that are known
up front (wq^T/32, wk^T, x^T, xq^T, plus x natural and wv) are prepared
host-side in bf16 and DMA'd as plain copies on the gpsimd/ACT queues,
weights first (they gate the A phase).  Output stores are bf16 on gpsimd.
The attention j-loop is software-pipelined with a 2-stage skew per stage
(S(s) | Px(s-2) | O(s-4)) so the PE never waits on exp->XBAR->matmul
chains; j=0 runs first so the drain ends on mid-size j=4.
PSUM-evictions run on the DVE; exp runs per 512-chunk on ACT with
accum_out providing softmax row-sums for free.
"""

import os

os.environ.setdefault("MYCRO_LOCAL_CACHE", "1")

import ml_dtypes
import numpy as np

import concourse.bacc as bacc
import concourse.tile as tile
from concourse import mybir
from concourse.bass_utils import run_bass_kernel_spmd

B, S, D = 4, 2048, 1024
P = 128
QL = S // 2          # queries per core
NCORES = 8
DT = D // P          # 8 d-tiles
ST = S // P          # 16 s-tiles
NQT = QL // P        # 8 q-tiles per core
F32 = mybir.dt.float32
BF16 = mybir.dt.bfloat16
NEG = -30000.0       # additive mask value; exp() underflows to exactly 0


def _chunks(extent):
    out, o = [], 0
    while o < extent:
        w = min(512, extent - o)
        out.append((o, w))
        o += w
    return out


def _body(tc, x, xt, xqt, wqt, wkt, wv, mask, out):
    nc = tc.nc
    with (
        tc.tile_pool(name="consts", bufs=1) as consts,
        tc.tile_pool(name="main", bufs=1) as main,
        tc.tile_pool(name="pmm", bufs=4, space="PSUM") as pmm,
        tc.tile_pool(name="psO", bufs=2, space="PSUM") as psO,
    ):
        mask_sb = consts.tile([P, 256], F32)

        xT = main.tile([P, DT, S], BF16)     # [d_in, d_tile, s]
        xqT = main.tile([P, DT, QL], BF16)   # [d_in, d_tile, q]
        xn = main.tile([P, ST, D], BF16)     # [s_in, s_tile, d]
        qa = main.tile([P, DT, QL], BF16)    # R^T = (xq A)^T : [d_in, d_tile, q]
        wv_sb = main.tile([P, DT, D], BF16)  # [d_in, d_tile, e]

        with tc.tile_pool(name="wscope", bufs=1) as ws:
            wqt_sb = ws.tile([P, DT, D], BF16)   # [e_in, e_tile, d']
            wkt_sb = ws.tile([P, DT, D], BF16)   # [e_in, e_tile, d]
            a_sb = ws.tile([P, DT, D], BF16)     # A: [d'_in, d'_tile, d]

            # All bulk input movement is plain copies (a DmaTranspose <->
            # DmaCopy transition serializes the whole DMA stream, so the only
            # device transposes are the per-j P^T/Px^T XBARs in attention).
            # Issue alternates gpsimd/scalar so descriptor-gen pipelines.
            # Order = need order: weights (gate A) -> xq^T -> x^T -> rest.
            def ld3(eng, dst, src):
                eng.dma_start(dst, src.rearrange("(t p) f -> p t f", p=P))

            for g in range(8):
                ld3(nc.gpsimd, wqt_sb[:, g:g + 1, :],
                    wqt[g * P:(g + 1) * P, :])
                ld3(nc.scalar, wkt_sb[:, g:g + 1, :],
                    wkt[g * P:(g + 1) * P, :])
            for g in range(2):
                ld3(nc.scalar, xqT[:, 4 * g:4 * g + 4, :],
                    xqt[g * 512:(g + 1) * 512, :])
            for g in range(4):
                ld3(nc.gpsimd, xT[:, 2 * g:2 * g + 2, :],
                    xt[g * 256:(g + 1) * 256, :])
            nc.scalar.dma_start(mask_sb, mask)
            for g in range(4):
                ld3((nc.gpsimd, nc.scalar)[g % 2], xn[:, 4 * g:4 * g + 4, :],
                    x[g * 512:(g + 1) * 512, :])
            for g in range(2):
                ld3((nc.gpsimd, nc.scalar)[g % 2], wv_sb[:, 4 * g:4 * g + 4, :],
                    wv[g * 512:(g + 1) * 512, :])

            # ---- HAM warmup: scratch matmuls (DVE-memset source, so they
            # start ~5us in) fill the weight-load window and lift the PE
            # clock gate to 8/8 before the real stream begins
            scr = consts.tile([P, 512], BF16)
            nc.vector.memset(scr, 0)
            for _ in range(12):
                pw = pmm.tile([P, 512], F32, tag="mm")
                nc.tensor.matmul(pw, scr[:, :P], scr, start=True, stop=True)

            # ---- A[d'tile, d] = sum_e wq[d',e]/32 wk[d,e]
            for ch in range(2):
                for dtp in range(DT):
                    ps = pmm.tile([P, 512], F32, tag="mm")
                    for et in range(DT):
                        nc.tensor.matmul(
                            ps, wqt_sb[:, et, dtp * P:(dtp + 1) * P],
                            wkt_sb[:, et, ch * 512:(ch + 1) * 512],
                            start=(et == 0), stop=(et == DT - 1))
                    nc.vector.tensor_copy(
                        a_sb[:, dtp, ch * 512:(ch + 1) * 512], ps)

            # ---- R^T[dtile, q] = sum_d' A[d', d] xq[q, d']
            for qc in (0, 1):        # qc=0 first: scores start at j=0
                for dt in range(DT):
                    ps = pmm.tile([P, 512], F32, tag="mm")
                    for dtp in range(DT):
                        nc.tensor.matmul(
                            ps, a_sb[:, dtp, dt * P:(dt + 1) * P],
                            xqT[:, dtp, qc * 512:(qc + 1) * 512],
                            start=(dtp == 0), stop=(dtp == DT - 1))
                    nc.vector.tensor_copy(
                        qa[:, dt, qc * 512:(qc + 1) * 512], ps)

        # ------------------------------ attention --------------------------
        with (
            tc.tile_pool(name="pp", bufs=3) as pp,
            tc.tile_pool(name="ptp", bufs=3) as ptp,
            tc.tile_pool(name="pxp", bufs=2) as pxp,
            tc.tile_pool(name="pxtp", bufs=3) as pxtp,
            tc.tile_pool(name="op", bufs=2) as op,
            tc.tile_pool(name="stats", bufs=5) as spool,
        ):
            st = {}

            def emit_scores(j):
                ext = 256 * (j + 1)
                nchunk = len(_chunks(ext))
                p_sb = pp.tile([P, S], BF16, tag="p")
                pt = ptp.tile([P, ST, P], BF16, tag="pt")
                lsum = spool.tile([P, 4], F32, tag="lsum")
                for ci, (o, w) in enumerate(_chunks(ext)):
                    ps = pmm.tile([P, 512], F32, tag="mm")
                    for dt in range(DT):
                        nc.tensor.matmul(
                            ps[:, :w], qa[:, dt, j * P:(j + 1) * P],
                            xT[:, dt, o:o + w],
                            start=(dt == 0), stop=(dt == DT - 1))
                    if o + w == ext:
                        nc.vector.tensor_add(
                            ps[:, w - 256:w], ps[:, w - 256:w], mask_sb)
                    nc.scalar.activation(
                        p_sb[:, o:o + w], ps[:, :w],
                        mybir.ActivationFunctionType.Exp,
                        accum_out=lsum[:, ci:ci + 1])
                nc.sync.dma_start(pt[:, :ext // P, :], p_sb[:, :ext],
                                  transpose=True)
                l_ = spool.tile([P, 1], F32, tag="l")
                nc.vector.reduce_sum(l_, lsum[:, :nchunk],
                                     axis=mybir.AxisListType.X)
                linv = spool.tile([P, 1], F32, tag="linv")
                nc.vector.reciprocal(linv, l_)
                st[j] = {"pt": pt, "linv": linv}

            def emit_px(j):
                nk = 2 * (j + 1)
                pt = st[j]["pt"]
                px_sb = pxp.tile([P, D], BF16, tag="px")
                pxt = pxtp.tile([P, DT, P], BF16, tag="pxt")
                for ec in range(2):
                    ps = pmm.tile([P, 512], F32, tag="mm")
                    for kt in range(nk):
                        nc.tensor.matmul(
                            ps, pt[:, kt, :], xn[:, kt, ec * 512:(ec + 1) * 512],
                            start=(kt == 0), stop=(kt == nk - 1))
                    nc.vector.tensor_copy(px_sb[:, ec * 512:(ec + 1) * 512], ps)
                nc.sync.dma_start(pxt[:, :, :], px_sb[:, :], transpose=True)
                st[j]["pxt"] = pxt

            def emit_o(j, nsplit=2):
                pxt = st[j]["pxt"]
                linv = st[j]["linv"]
                po = psO.tile([P, D], F32, tag="o")
                for ec in range(2):
                    for dt in range(DT):
                        nc.tensor.matmul(
                            po[:, ec * 512:(ec + 1) * 512], pxt[:, dt, :],
                            wv_sb[:, dt, ec * 512:(ec + 1) * 512],
                            start=(dt == 0), stop=(dt == DT - 1))
                o_sb = op.tile([P, D], BF16, tag="osb")
                w = D // nsplit
                for pc in range(nsplit):
                    nc.vector.tensor_scalar_mul(
                        o_sb[:, pc * w:(pc + 1) * w],
                        po[:, pc * w:(pc + 1) * w], linv)
                    # HWDGE (scalar) store: the SWDGE end-of-kernel ring
                    # drain costs ~3.5us, the HWDGE one is ~free
                    nc.scalar.dma_start(
                        out[j * P:(j + 1) * P, pc * w:(pc + 1) * w],
                        o_sb[:, pc * w:(pc + 1) * w])
                del st[j]

            # j=0 first so the pipeline drains on a mid-size j (its long Px
            # hides the final PxT XBAR latency); big/small interleave keeps
            # short stages' serialized PT/PxT/out DMA chains hidden.
            # skew-2 between stages: every XBAR gets two stages to land.
            js = [0, 7, 3, 6, 2, 5, 1, 4]
            for step in range(len(js) + 4):
                if step < len(js):
                    emit_scores(js[step])
                if 2 <= step < len(js) + 2:
                    emit_px(js[step - 2])
                if step >= 4:
                    emit_o(js[step - 4], nsplit=4 if step == len(js) + 3 else 2)


_PROG = None


def _get_prog():
    global _PROG
    if _PROG is None:
        nc = bacc.Bacc("TRN2", target_bir_lowering=False, debug=False,
                       enable_asserts=False)
        x = nc.dram_tensor("x", (S, D), BF16, kind="ExternalInput").ap()
        xt = nc.dram_tensor("xt", (D, S), BF16, kind="ExternalInput").ap()
        xqt = nc.dram_tensor("xqt", (D, QL), BF16, kind="ExternalInput").ap()
        wqt = nc.dram_tensor("wqt", (D, D), BF16, kind="ExternalInput").ap()
        wkt = nc.dram_tensor("wkt", (D, D), BF16, kind="ExternalInput").ap()
        wv = nc.dram_tensor("wv", (D, D), BF16, kind="ExternalInput").ap()
        mask = nc.dram_tensor("mask", (P, 256), F32, kind="ExternalInput").ap()
        out = nc.dram_tensor("out", (QL, D), BF16, kind="ExternalOutput").ap()
        with tile.TileContext(nc) as tc:
            _body(tc, x, xt, xqt, wqt, wkt, wv, mask, out)
        nc.compile()
        _PROG = nc
    return _PROG


def _mask_np(h):
    r = np.arange(P)[:, None]
    c = np.arange(P)[None, :]
    tri = np.where(c <= r, 0.0, NEG).astype(np.float32)
    m = np.zeros((P, 256), np.float32)
    if h == 0:
        m[:, :P] = tri
        m[:, P:] = NEG
    else:
        m[:, P:] = tri
    return m


def _prep_shared(inputs):
    bf = ml_dtypes.bfloat16
    wq = np.asarray(inputs["wq"], np.float32)
    wk = np.asarray(inputs["wk"], np.float32)
    wv = np.asarray(inputs["wv"], np.float32)
    return {
        "wqt": np.ascontiguousarray((wq / 32.0).astype(bf).T),
        "wkt": np.ascontiguousarray(wk.astype(bf).T),
        "wv": np.ascontiguousarray(wv.astype(bf)),
    }


def _in_map_for_core(inputs, core, shared=None):
    b, h = core // 2, core % 2
    if shared is None:
        shared = _prep_shared(inputs)
    xb = np.ascontiguousarray(
        np.asarray(inputs["x"], np.float32)[b].astype(ml_dtypes.bfloat16))
    xqb = xb.reshape(NQT, 2, P, D)[:, h].reshape(QL, D)
    return {"x": xb, "xt": np.ascontiguousarray(xb.T),
            "xqt": np.ascontiguousarray(xqb.T), "mask": _mask_np(h), **shared}


def _run(inputs, trace=False, tmpdir=None):
    nc = _get_prog()
    shared = _prep_shared(inputs)
    in_maps = [_in_map_for_core(inputs, c, shared) for c in range(NCORES)]
    res = None
    for attempt in range(3):
        try:
            res = run_bass_kernel_spmd(nc, in_maps,
                                       core_ids=list(range(NCORES)),
                                       trace=trace, tmpdir=tmpdir)
            break
        except Exception:
            # first execution of a fresh NEFF occasionally trips a transient
            # device error on this stack; a retry has always succeeded
            if attempt == 2:
                raise
    outf = np.empty((B, S, D), np.float32)
    for core in range(NCORES):
        b, h = core // 2, core % 2
        o = np.asarray(res.results[core]["out"], np.float32)
        outf[b].reshape(NQT, 2, P, D)[:, h] = o.reshape(NQT, P, D)
    return outf, res


def kernel(x, wq, wk, wv):
    outf, _ = _run({"x": x, "wq": wq, "wk": wk, "wv": wv}, trace=False)
    return outf

